# revision 6
# baseline (speedup 1.0000x reference)
"""Trainium2 Bass kernel for XL-memory MQA attention (8 NeuronCores).

Sharding: tensor-parallel over heads (2 heads/core, K/V replicated), per the
MQA structure. Each core computes q/k/v projections from x, the attention for
its 2 heads over all 4 batches, and a partial output projection (its 128 rows
of Wout); the host sums the 8 partial outputs.

Device-side structure (per core):
  - projections run in float32r (full PE speed, ~1e-4 matmul precision)
  - sim is computed transposed [j, i] = K @ qT so that softmax weights feed the
    attn@V matmul as natural bf16 operands with K=128 contraction
  - rel_pos_bias (pre-masked with -1e30, bf16) is copied into PSUM by an
    identity matmul opening each accumulation group; the two heads' K=64 sim
    matmuls then run concurrently via tile_position row pairing
  - softmax skips the max-subtraction (inputs are ~N(0,2); exp is safe in f32)
    and obtains row sums from a ones-column appended to V
  - fully-masked j-tiles (j0 > i0 + 1023) are skipped entirely
"""
import numpy as np
import ml_dtypes

import concourse.bacc as bacc
import concourse.mybir as mybir
import concourse.tile as tile
from concourse import masks
from concourse.bass_utils import run_bass_kernel_spmd

F32 = mybir.dt.float32
F32R = mybir.dt.float32r
BF16 = mybir.dt.bfloat16
Act = mybir.ActivationFunctionType
BF = ml_dtypes.bfloat16

B = 4          # batch
N = 2048       # query tokens
D = 1024       # model dim
M = 512        # xl memory tokens
J = N + M      # key tokens (2560)
DH = 64        # head dim
NCORES = 8
NEG = -1e30

JT = J // 128          # 20 j-tiles of 128
IC = N // 512          # 4 i-chunks of 512


def _jt_max(ic):
    # query chunk [ic*512, ic*512+511] attends keys j <= i + 512
    # tile [j0, j0+127] fully masked iff j0 > (ic*512+511) + 512
    return min(JT, (ic * 512 + 1023) // 128 + 1)   # 8, 12, 16, 20


def _build():
    nc = bacc.Bacc("TRN2", target_bir_lowering=False, debug=False,
                   num_devices=NCORES)
    xT = nc.dram_tensor("xT", [B, D, N], F32R, kind="ExternalInput").ap()
    wq = nc.dram_tensor("wq", [128, 8, 128], F32R, kind="ExternalInput").ap()
    wkv = nc.dram_tensor("wkv", [128, 8, 128], F32R, kind="ExternalInput").ap()
    wout = nc.dram_tensor("wout", [128, D], BF16, kind="ExternalInput").ap()
    biasT = nc.dram_tensor("biasT", [2, JT, 128, N], BF16,
                           kind="ExternalInput").ap()
    xlkT = nc.dram_tensor("xlkT", [B, DH, M], BF16, kind="ExternalInput").ap()
    xlv = nc.dram_tensor("xlv", [B, 128, 4, DH], BF16,
                         kind="ExternalInput").ap()
    yT = nc.dram_tensor("yT", [B, D, N], F32, kind="ExternalOutput").ap()
    newkT = nc.dram_tensor("newkT", [B, DH, M], F32, kind="ExternalOutput").ap()
    newvT = nc.dram_tensor("newvT", [B, DH, M], F32, kind="ExternalOutput").ap()

    with tile.TileContext(nc) as tc:
        with tc.tile_pool(name="res", bufs=1) as res, \
             tc.tile_pool(name="xin", bufs=8) as xin, \
             tc.tile_pool(name="bias", bufs=40) as biasp, \
             tc.tile_pool(name="awp", bufs=3) as awp, \
             tc.tile_pool(name="small", bufs=2) as small, \
             tc.tile_pool(name="rbcp", bufs=2) as rbcp, \
             tc.tile_pool(name="normp", bufs=3) as normp, \
             tc.tile_pool(name="ytp", bufs=2) as ytp, \
             tc.tile_pool(name="ps", bufs=2, space="PSUM") as ps:

            ident = res.tile([128, 128], BF16)
            masks.make_identity(nc, ident[:])

            wq_sb = res.tile([128, 8, 128], F32R)
            wkv_sb = res.tile([128, 8, 128], F32R)
            wout_sb = res.tile([128, D], BF16)
            nc.sync.dma_start(wq_sb[:], wq[:])
            nc.sync.dma_start(wkv_sb[:], wkv[:])
            nc.sync.dma_start(wout_sb[:], wout[:])

            qT_sb = res.tile([128, B * N], BF16)       # 2 heads x 64 dims
            kT_sb = res.tile([128, B * J], BF16)       # duplicated halves
            vaug = [res.tile([128, JT, DH + 1], BF16, name=f"vaug{b}")
                    for b in range(B)]
            vtt = res.tile([DH, N], BF16, bufs=2)      # vT staging, per batch
            attnraw = res.tile([65, B * IC * 1024], BF16)

            # ---------------- Phase 1: projections, per batch ----------------
            for b in range(B):
                nc.sync.dma_start(kT_sb[0:DH, b * J: b * J + M], xlkT[b])
                nc.sync.dma_start(kT_sb[64:64 + DH, b * J: b * J + M], xlkT[b])
                nc.sync.dma_start(vaug[b][:, 0:4, 0:DH], xlv[b])
                for tt in range(4):
                    xts = []
                    for ci in range(8):
                        xt_t = xin.tile([128, 512], F32R, tag="xt",
                                        name=f"xt{b}_{tt}_{ci}")
                        nc.sync.dma_start(
                            xt_t[:], xT[b, ci * 128:(ci + 1) * 128,
                                        tt * 512:(tt + 1) * 512])
                        xts.append(xt_t)
                    qps = ps.tile([128, 512], F32, tag="sim",
                                  padded_shape=[128, 1024])
                    kvps = ps.tile([128, 512], F32, tag="acc",
                                   padded_shape=[128, 1024])
                    for ci in range(8):
                        nc.tensor.matmul(qps[:], wq_sb[:, ci, :], xts[ci][:],
                                         start=(ci == 0), stop=(ci == 7))
                        nc.tensor.matmul(kvps[:], wkv_sb[:, ci, :], xts[ci][:],
                                         start=(ci == 0), stop=(ci == 7))
                    nc.scalar.activation(
                        qT_sb[:, b * N + tt * 512: b * N + (tt + 1) * 512],
                        qps[:], Act.Copy)
                    nc.scalar.activation(
                        kT_sb[0:DH, b * J + M + tt * 512:
                              b * J + M + (tt + 1) * 512],
                        kvps[0:DH, :], Act.Copy)
                    nc.scalar.activation(
                        vtt[:, tt * 512:(tt + 1) * 512],
                        kvps[64:64 + DH, :], Act.Copy)
                    if tt == 3:
                        nkv = small.tile([128, 512], F32, tag="nkv")
                        nc.scalar.activation(nkv[0:DH, :], kvps[0:DH, :],
                                             Act.Copy)
                        nc.vector.tensor_copy(nkv[64:128, :],
                                              kvps[64:64 + DH, :])
                        nc.sync.dma_start(newkT[b], nkv[0:DH, :])
                        nc.sync.dma_start(newvT[b], nkv[64:128, :])
                # duplicate computed kT to lower partition half
                nc.sync.dma_start(kT_sb[64:128, b * J + M: (b + 1) * J],
                                  kT_sb[0:DH, b * J + M: (b + 1) * J])
                # transpose vT -> V rows (j-tiles 4..19)
                for g in range(4):
                    tp = ps.tile([128, 256], BF16, tag="sim",
                                 padded_shape=[128, 2048], name=f"tp{b}_{g}")
                    for k in range(4):
                        nc.tensor.transpose(
                            tp[:, k * 64:(k + 1) * 64],
                            vtt[:, (g * 4 + k) * 128:(g * 4 + k + 1) * 128],
                            ident[0:DH, 0:DH])
                    nc.scalar.activation(
                        vaug[b][:, 4 + g * 4: 8 + g * 4, 0:DH], tp[:],
                        Act.Copy)
                nc.gpsimd.memset(vaug[b][:, :, DH:DH + 1], 1.0)

            # ---------------- Phase 2: attention ----------------
            def normalize_outproj(ic, b, stg):
                base = (b * IC + ic) * 1024
                rbcA = rbcp.tile([128, 512], BF16, tag="rbc",
                                 name=f"rbcA{ic}_{b}")
                rbcB = rbcp.tile([128, 512], BF16, tag="rbc",
                                 name=f"rbcB{ic}_{b}")
                nc.gpsimd.partition_broadcast(
                    rbcA[:], stg[0:1, (b * 2) * 512:(b * 2 + 1) * 512])
                nc.gpsimd.partition_broadcast(
                    rbcB[:], stg[0:1, (b * 2 + 1) * 512:(b * 2 + 2) * 512])
                st2 = normp.tile([128, 512], BF16, tag="st2",
                                 name=f"st2_{ic}_{b}")
                nc.sync.dma_start(st2[64:128, :],
                                  attnraw[0:64, base + 512: base + 1024])
                norm = normp.tile([128, 512], BF16, tag="norm",
                                  name=f"norm{ic}_{b}")
                nc.vector.tensor_mul(norm[0:64, :],
                                     attnraw[0:64, base: base + 512],
                                     rbcA[0:64, :])
                nc.vector.tensor_mul(norm[64:128, :], st2[64:128, :],
                                     rbcB[64:128, :])
                for yc in range(8):
                    yp = ps.tile([128, 512], F32, tag="sim",
                                 padded_shape=[128, 1024],
                                 name=f"yp{ic}_{b}_{yc}")
                    nc.tensor.matmul(yp[:], wout_sb[:, yc * 128:(yc + 1) * 128],
                                     norm[:], start=True, stop=True)
                    yt_t = ytp.tile([128, 512], F32, tag="yt",
                                    name=f"yt{ic}_{b}_{yc}")
                    nc.vector.tensor_copy(yt_t[:], yp[:])
                    nc.sync.dma_start(
                        yT[b, yc * 128:(yc + 1) * 128,
                           ic * 512:(ic + 1) * 512], yt_t[:])

            stgs = {}
            for ic in range(IC):
                jmax = _jt_max(ic)
                btiles = {}
                for h in range(2):
                    for jt in range(jmax):
                        bt = biasp.tile([128, 512], BF16, tag="bias",
                                        name=f"bt{ic}_{h}_{jt}")
                        nc.sync.dma_start(
                            bt[:], biasT[h, jt, :, ic * 512:(ic + 1) * 512])
                        btiles[h, jt] = bt
                for b in range(B):
                    qA = qT_sb[0:64, b * N + ic * 512: b * N + (ic + 1) * 512]
                    qB = qT_sb[64:128, b * N + ic * 512: b * N + (ic + 1) * 512]
                    acc = ps.tile([128, 1024], F32, tag="acc",
                                  name=f"acc{ic}_{b}")
                    for jt in range(jmax):
                        sim = ps.tile([128, 1024], F32, tag="sim",
                                      name=f"sim{ic}_{b}_{jt}")
                        nc.tensor.matmul(sim[:, 0:512], ident[:],
                                         btiles[0, jt][:],
                                         start=True, stop=False,
                                         skip_group_check=True)
                        nc.tensor.matmul(sim[:, 512:1024], ident[:],
                                         btiles[1, jt][:],
                                         start=True, stop=False,
                                         skip_group_check=True)
                        kslice = slice(b * J + jt * 128, b * J + (jt + 1) * 128)
                        nc.tensor.matmul(sim[:, 0:512], kT_sb[0:64, kslice],
                                         qA, start=False, stop=True,
                                         tile_position=(0, 0),
                                         skip_group_check=True)
                        nc.tensor.matmul(sim[:, 512:1024],
                                         kT_sb[64:128, kslice],
                                         qB, start=False, stop=True,
                                         tile_position=(64, 0),
                                         skip_group_check=True)
                        aw = awp.tile([128, 1024], BF16, tag="aw",
                                      name=f"aw{ic}_{b}_{jt}")
                        nc.scalar.activation(aw[:], sim[:], Act.Exp)
                        nc.tensor.matmul(acc[0:65, 0:512], vaug[b][:, jt, :],
                                         aw[:, 0:512], start=(jt == 0),
                                         stop=(jt == jmax - 1),
                                         skip_group_check=True)
                        nc.tensor.matmul(acc[0:65, 512:1024],
                                         vaug[b][:, jt, :],
                                         aw[:, 512:1024], start=(jt == 0),
                                         stop=(jt == jmax - 1),
                                         skip_group_check=True)
                    nc.scalar.activation(
                        attnraw[0:65, (b * IC + ic) * 1024:
                                (b * IC + ic + 1) * 1024],
                        acc[0:65, :], Act.Copy)
                    # interleave previous i-chunk's output projection here so
                    # the PE queue never waits on the recip chain
                    if ic >= 1:
                        normalize_outproj(ic - 1, b, stgs[ic - 1])
                # reciprocal of the softmax sums for this i-chunk (all b, h)
                rin = small.tile([8, 512], BF16, tag="rin", name=f"rin{ic}")
                ar5 = attnraw[64:65, :].rearrange(
                    "p (b i h f) -> p b i h f", b=B, i=IC, h=2)
                for b in range(B):
                    nc.sync.dma_start(rin[b * 2:(b + 1) * 2, :],
                                      ar5[:, b, ic, :, :])
                rout = small.tile([8, 512], BF16, tag="rout", name=f"rout{ic}")
                with nc.allow_low_precision(reason="softmax denom in bf16"):
                    nc.vector.reciprocal(rout[:], rin[:])
                stg = small.tile([1, 4096], BF16, tag="stg", name=f"stg{ic}",
                                 bufs=1)
                nc.sync.dma_start(
                    stg[:].rearrange("a (p f) -> a p f", p=8),
                    rout[:].unsqueeze(1))
                stgs[ic] = stg
            for b in range(B):
                normalize_outproj(IC - 1, b, stgs[IC - 1])

    nc.compile()
    return nc


_CACHE = {}


def _get_nc():
    if "nc" not in _CACHE:
        _CACHE["nc"] = _build()
    return _CACHE["nc"]


def kernel(x, xl_memory, rel_pos_bias, Wq, Wkv, Wout, b_out):
    x = np.asarray(x, dtype=np.float32)
    xl_memory = np.asarray(xl_memory, dtype=np.float32)
    rel_pos_bias = np.asarray(rel_pos_bias, dtype=np.float32)
    Wq = np.asarray(Wq, dtype=np.float32)
    Wkv = np.asarray(Wkv, dtype=np.float32)
    Wout = np.asarray(Wout, dtype=np.float32)
    b_out = np.asarray(b_out, dtype=np.float32)

    scale = DH ** -0.5
    xT = np.ascontiguousarray(x.transpose(0, 2, 1))                 # [B, D, N]
    wkv_t = np.ascontiguousarray(
        Wkv.reshape(8, 128, 128).transpose(1, 0, 2))                # [128,8,128]
    xlkT = np.ascontiguousarray(
        xl_memory[..., 0, :].transpose(0, 2, 1)).astype(BF)         # [B,DH,M]
    xlv = np.ascontiguousarray(
        xl_memory[..., 1, :].reshape(B, 4, 128, DH)
        .transpose(0, 2, 1, 3)).astype(BF)                          # [B,128,4,DH]

    # causal mask in transposed (j, i) coords: masked where j > i + M
    jj = np.arange(J)[:, None]
    ii = np.arange(N)[None, :]
    maskT = jj > (ii + M)

    in_maps = []
    for c in range(NCORES):
        wq_c = np.ascontiguousarray(
            (Wq[:, c * 128:(c + 1) * 128] * scale)
            .reshape(8, 128, 128).transpose(1, 0, 2))
        wout_c = np.ascontiguousarray(
            Wout[c * 128:(c + 1) * 128, :]).astype(BF)
        bt = np.empty((2, JT, 128, N), dtype=BF)
        for h in range(2):
            bh = rel_pos_bias[c * 2 + h].T.copy()                   # [J, N]
            bh[maskT] = NEG
            bt[h] = bh.reshape(JT, 128, N).astype(BF)
        in_maps.append({
            "xT": xT, "wq": wq_c, "wkv": wkv_t, "wout": wout_c,
            "biasT": bt, "xlkT": xlkT, "xlv": xlv,
        })

    nc = _get_nc()
    res = run_bass_kernel_spmd(nc, in_maps, core_ids=list(range(NCORES)),
                               trace=False)
    _CACHE["last_res"] = res

    yT_sum = np.zeros((B, D, N), dtype=np.float32)
    for c in range(NCORES):
        yT_sum += res.results[c]["yT"]
    out = yT_sum.transpose(0, 2, 1) + b_out                         # [B, N, D]

    r0 = res.results[0]
    new_k = r0["newkT"].transpose(0, 2, 1)                          # [B, M, DH]
    new_v = r0["newvT"].transpose(0, 2, 1)
    new_xl = np.stack([new_k, new_v], axis=2)                       # [B,M,2,DH]
    return out.astype(np.float32), new_xl.astype(np.float32)
